# revision 1
# baseline (speedup 1.0000x reference)
"""Trainium2 Bass kernel for nn_AFM (attentional factorization machine).

Mathematical reduction (validated against the reference):
  - softmax over a size-1 axis == 1, so the attention MLP is dead code and
    fAtt = mean(fPI, axis=1).
  - FM identity per (b, m): sum_{i<j} x_i x_j = ((sum_i x_i)^2 - sum_i x_i^2)/2
    with x_i = dense[b,i,m] * v[i,m].
  With S1[b,m] = sum_n dense[b,n,m] v[n,m], S2[b,m] = sum_n (dense[b,n,m] v[n,m])^2,
  c[m] = Wp[m] / (2 * P):
    out[b] = sum_n dense[b,n,0] Wl[n] + bl + bp + sum_m c[m] (S1[b,m]^2 - S2[b,m])

Sharding: pure data parallel, batch 4096 -> 512 rows on each of 8 cores.

Raw-bass SPMD program (no Tile framework; manual semaphores) per core,
software-pipelined over four 128-row tiles, each loaded in two half-tiles:
  SYNC: HWDGE f32 half-tile loads + small param loads; one batched [128,4]
        output store at the end (host transposes to batch order).
  DVE:  per tile: dva/dvb = d*v on each half (f32 in, bf16 out), S1 log-tree
        (level 1 sums the halves), then - pipelined one tile behind - the S2
        log-tree over the squared halves and the fused combine chain
        (custom-DVE tensor-tensor-reduce). GpSimd is deliberately unused:
        concurrent GpSimd traffic knocks DVE tensor ops out of their 2x mode.
  ACT:  per tile: square(dva), square(dvb) -> bf16 halves for the S2 tree.
Cross-engine ordering uses per-engine chain semaphores; every compute
instruction waits on its chain and increments it. Cross-engine wait
thresholds are chosen so each semaphore value has a unique producer
(required by the race checker); WAIT_OVERRIDES carries sim-calibrated
adjustments.
"""

import numpy as np

B, N, M = 4096, 32, 64
NM = N * M                  # 2048
HALF = NM // 2              # 1024 (= n in [0,16) block)
NCORES = 8
BS = B // NCORES            # 512 rows per core
TILES = BS // 128           # 4 tiles of 128 batch rows per core
P_PAIRS = N * (N - 1) // 2  # 496

# tree level output widths: 1024 (sum of halves), then halving to 64
LVLS = [1024, 512, 256, 128, 64]

_CACHE = {}

WAIT_OVERRIDES = {('vch', 5): 6}  # sim-calibrated


def _build_program():
    from concourse import bacc, mybir
    from concourse.dve_ops import TENSOR_TENSOR_REDUCE as CTTR

    f32 = mybir.dt.float32
    bf16 = mybir.dt.bfloat16

    nc = bacc.Bacc("TRN2", target_bir_lowering=False, debug=False)
    dense = nc.declare_dram_parameter("dense", [BS, NM], f32, isOutput=False)
    vrep = nc.declare_dram_parameter("vrep", [128, NM], bf16, isOutput=False)
    crep = nc.declare_dram_parameter("crep", [128, M], f32, isOutput=False)
    wlrep = nc.declare_dram_parameter("wlrep", [128, N], f32, isOutput=False)
    cst = nc.declare_dram_parameter("cst", [128, 1], f32, isOutput=False)
    out = nc.declare_dram_parameter("out", [128, TILES], f32, isOutput=True)

    sb = lambda name, shape, dt: nc.alloc_sbuf_tensor(name, list(shape), dt)

    vrep_t = sb("vrep_t", [128, NM], bf16)
    crep_t = sb("crep_t", [128, M], f32)
    wlrep_t = sb("wlrep_t", [128, N], f32)
    cst_t = sb("cst_t", [128, 1], f32)
    o2all = sb("o2all", [128, TILES], f32)

    # ds*_t hold [dv | sq] side by side: DVE writes cols [0:HALF] (d*v),
    # ACT writes cols [HALF:2*HALF] (square of the dv half). The two
    # reduction trees then run as combined ops over both halves.
    df_t, dsa_t, dsb_t = [], [], []
    s12lv = []
    cs1_t, junkM, junkN, pc1_t, pc2_t = [], [], [], [], []
    for t in range(TILES):
        df_t.append(sb(f"df{t}", [128, NM], f32))
        dsa_t.append(sb(f"dsa{t}", [128, NM], bf16))
        dsb_t.append(sb(f"dsb{t}", [128, NM], bf16))
        s12lv.append(
            [sb(f"s12_{t}_{w}", [128, 2 * w], f32 if w == M else bf16) for w in LVLS]
        )
        cs1_t.append(sb(f"cs1_{t}", [128, M], f32))
        junkM.append(sb(f"junkM_{t}", [128, M], f32))
        junkN.append(sb(f"junkN_{t}", [128, N], f32))
        pc1_t.append(sb(f"pc1_{t}", [128, 1], f32))
        pc2_t.append(sb(f"pc2_{t}", [128, 1], f32))

    def tree_step(eng, t, lvl):
        """One combined halving add over both trees ([s1-block | s2-block])."""
        levels = s12lv[t]
        w = LVLS[lvl]
        if lvl == 0:
            return eng.tensor_add(levels[0].ap(), dsa_t[t].ap(), dsb_t[t].ap())
        src = levels[lvl - 1].ap().rearrange("p (s w) -> p s w", s=2)
        return eng.tensor_add(
            levels[lvl].ap().rearrange("p (s w) -> p s w", s=2),
            src[:, :, 0:w],
            src[:, :, w : 2 * w],
        )

    cnt = {"v": 0, "a": 0, "s": 0}
    chains = {}

    def emit(e, ins):
        ins._wait_ge(chains[e], cnt[e]).then_inc(chains[e], 1)
        cnt[e] += 1
        return cnt[e]

    def emit_dma(e, ins, sem, inc, wait=None):
        if wait is not None:
            wsem, wval = wait
            wval = WAIT_OVERRIDES.get((wsem.name, wval), wval)
            ins._wait_ge(wsem, wval)
        else:
            ins._wait_ge(chains[e], cnt[e])
        ins.then_inc(sem, inc)

    def emit_wait(e, eng, sem, val):
        val = WAIT_OVERRIDES.get((sem.name, val), val)
        eng.wait_ge(sem, val).then_inc(chains[e], 1)
        cnt[e] += 1

    dv_done = [0] * TILES      # vchain value after dvb of tile t
    sq_done = [0] * TILES      # achain value after sqb of tile t
    s2first_done = [0] * TILES # vchain value after first s2 tree op of tile t
    o2_done = [0] * TILES      # vchain value after final combine of tile t

    with (
        nc.Block() as block,
        nc.semaphore("vch") as vch,
        nc.semaphore("ach") as ach,
        nc.semaphore("sch") as sch,
        nc.semaphore("ld0a") as ld0a,
        nc.semaphore("ld0b") as ld0b,
        nc.semaphore("ld1a") as ld1a,
        nc.semaphore("ld1b") as ld1b,
        nc.semaphore("ld2a") as ld2a,
        nc.semaphore("ld2b") as ld2b,
        nc.semaphore("ld3a") as ld3a,
        nc.semaphore("ld3b") as ld3b,
        nc.semaphore("prm") as prm,
        nc.semaphore("sts") as sts,
    ):
        chains.update(v=vch, a=ach, s=sch)
        lda = [ld0a, ld1a, ld2a, ld3a]
        ldb = [ld0b, ld1b, ld2b, ld3b]

        @block.vector
        def _(dve):
            def head(t):
                emit_wait("v", dve, lda[t], 16)
                emit("v", dve.tensor_mul(
                    dsa_t[t].ap()[:, 0:HALF], df_t[t].ap()[:, 0:HALF],
                    vrep_t.ap()[:, 0:HALF],
                ))
                emit_wait("v", dve, ldb[t], 16)
                dv_done[t] = emit("v", dve.tensor_mul(
                    dsb_t[t].ap()[:, 0:HALF], df_t[t].ap()[:, HALF:NM],
                    vrep_t.ap()[:, HALF:NM],
                ))

            def tail(t):
                # ach counts 3 per tile (wait, sqa, sqb); scalar block is
                # built after this one so sq_done[t] isn't available yet
                emit_wait("v", dve, ach, 3 * (t + 1))
                s2first_done[t] = cnt["v"] + 1  # vch value of combined L1
                for lvl in range(len(LVLS)):
                    emit("v", tree_step(dve, t, lvl))
                s1f = s12lv[t][-1].ap()[:, 0:M]
                s2f = s12lv[t][-1].ap()[:, M : 2 * M]
                emit("v", dve.tensor_mul(cs1_t[t].ap(), s1f, crep_t.ap()))
                emit("v", dve._custom_dve(
                    CTTR, out=junkM[t].ap(), in0=cs1_t[t].ap(),
                    in1=s1f, s0=cst_t.ap(), s1=1.0,
                    accum_out=pc1_t[t].ap(),
                ))
                emit("v", dve._custom_dve(
                    CTTR, out=junkM[t].ap(), in0=s2f,
                    in1=crep_t.ap(), s0=pc1_t[t].ap(), s1=-1.0,
                    accum_out=pc2_t[t].ap(),
                ))
                d_col0 = (
                    df_t[t]
                    .ap()
                    .rearrange("p (n m) -> p n m", n=N)[:, :, 0:1]
                    .rearrange("p n one -> p (n one)")
                )
                o2_done[t] = emit("v", dve._custom_dve(
                    CTTR, out=junkN[t].ap(), in0=d_col0, in1=wlrep_t.ap(),
                    s0=pc2_t[t].ap(), s1=1.0,
                    accum_out=o2all.ap()[:, t : t + 1],
                ))

            # heads (multiplies) interleave with tails (trees+combine),
            # hiding the square latency behind the next tile's multiplies
            emit_wait("v", dve, prm, 64)
            head(0)
            head(1)
            tail(0)
            head(2)
            tail(1)
            head(3)
            tail(2)
            tail(3)

        @block.scalar
        def _(act):
            # param loads ride the Activation HWDGE ring so they don't
            # compete with the dense loads on the SP ring
            emit_dma(
                "a",
                act.dma_start(
                    out=df_t[0].ap()[:, HALF:NM], in_=dense.ap()[0:128, HALF:NM]
                ),
                ldb[0], 16,
            )
            emit_dma("a", act.dma_start(out=crep_t.ap(), in_=crep.ap()), prm, 16)
            emit_dma("a", act.dma_start(out=wlrep_t.ap(), in_=wlrep.ap()), prm, 16)
            emit_dma("a", act.dma_start(out=cst_t.ap(), in_=cst.ap()), prm, 16)
            for t in range(TILES):
                # Pin the ach increment order: gate on tail(t-1)'s combined
                # L1 (which requires sq(t-1)); it also covers dv_t since
                # tail(t-1) follows head(t) in the DVE stream. t=0 gates on
                # its own dvb.
                thr = dv_done[t] if t == 0 else s2first_done[t - 1]
                emit_wait("a", act, vch, thr)
                emit("a", act.square(
                    dsa_t[t].ap()[:, HALF:NM], dsa_t[t].ap()[:, 0:HALF]
                ))
                sq_done[t] = emit("a", act.square(
                    dsb_t[t].ap()[:, HALF:NM], dsb_t[t].ap()[:, 0:HALF]
                ))

        @block.sync
        def _(sync):
            def ld(t, h):
                lo, hi = (0, HALF) if h == 0 else (HALF, NM)
                emit_dma(
                    "s",
                    sync.dma_start(
                        out=df_t[t].ap()[:, lo:hi],
                        in_=dense.ap()[128 * t : 128 * (t + 1), lo:hi],
                    ),
                    (lda if h == 0 else ldb)[t], 16,
                )

            # throttle: only one tile's loads queued at a time, so each
            # load's completion semaphore fires as soon as its data lands
            # (a deep queue round-robins packets and delays the first
            # completion to nearly the last)
            ld(0, 0)
            emit_dma("s", sync.dma_start(out=vrep_t.ap(), in_=vrep.ap()), prm, 16)
            for t in range(1, TILES):
                emit_wait("s", sync, lda[t - 1], 16)
                ld(t, 0)
                ld(t, 1)
            emit_dma(
                "s",
                sync.dma_start(out=out.ap(), in_=o2all.ap()),
                sts, 16,
                wait=(vch, o2_done[3]),
            )
            sync.wait_ge(sts, 16)

    nc.compile()
    return nc


def _get_program():
    if "nc" not in _CACHE:
        _CACHE["nc"] = _build_program()
    return _CACHE["nc"]


def _host_prep(inputs):
    dense = np.ascontiguousarray(
        np.asarray(inputs["dense"], dtype=np.float32).reshape(B, NM)
    )
    v = np.asarray(inputs["v"], dtype=np.float32).reshape(1, NM)
    Wl = np.asarray(inputs["Wl"], dtype=np.float32).reshape(N)
    Wp = np.asarray(inputs["Wp"], dtype=np.float32).reshape(M)
    bl = float(np.asarray(inputs["bl"], dtype=np.float32).reshape(-1)[0])
    bp = float(np.asarray(inputs["bp"], dtype=np.float32).reshape(-1)[0])

    import ml_dtypes

    c = (Wp / (2.0 * P_PAIRS)).astype(np.float32)
    vrep = np.ascontiguousarray(
        np.broadcast_to(v.astype(ml_dtypes.bfloat16), (128, NM))
    )
    crep = np.ascontiguousarray(np.broadcast_to(c[None, :], (128, M)))
    wlrep = np.ascontiguousarray(np.broadcast_to(Wl[None, :], (128, N)))
    cst = np.full((128, 1), bl + bp, dtype=np.float32)

    in_maps = []
    for i in range(NCORES):
        in_maps.append(
            {
                "dense": dense[BS * i : BS * (i + 1)],
                "vrep": vrep,
                "crep": crep,
                "wlrep": wlrep,
                "cst": cst,
            }
        )
    return in_maps


def _gather(res):
    # out[p, t] holds batch row 128*t + p of the core's shard
    outs = []
    for i in range(NCORES):
        arr = np.asarray(res.results[i]["out"], np.float32)  # [128, TILES]
        outs.append(arr.T.reshape(BS))
    return np.concatenate(outs).reshape(B, 1)


def kernel(**inputs) -> np.ndarray:
    from concourse.bass_utils import run_bass_kernel_spmd

    nc = _get_program()
    in_maps = _host_prep(inputs)
    res = run_bass_kernel_spmd(nc, in_maps, core_ids=list(range(NCORES)))
    return _gather(res)



# revision 2
# speedup vs baseline: 1.2282x; 1.2282x over previous
"""Trainium2 Bass kernel for nn_AFM (attentional factorization machine).

Mathematical reduction (validated against the reference):
  - softmax over a size-1 axis == 1, so the attention MLP is dead code and
    fAtt = mean(fPI, axis=1).
  - FM identity per (b, m): sum_{i<j} x_i x_j = ((sum_i x_i)^2 - sum_i x_i^2)/2
    with x_i = dense[b,i,m] * v[i,m].
  - Sign-split scaling: with c[m] = Wp[m]/(2P), u[n,m] = v[n,m]*sqrt(|c[m]|)
    and y = d*u, the FM term becomes
      sum_m sign(c[m]) * ((sum_n y)^2 - sum_n y^2).
    Host reorders the m axis so all c>=0 columns come first (K of them);
    then sum_m sign*(sum_n y^2) collapses to TWO plain free-axis sums of y^2
    (one per contiguous sign block) - computed on the otherwise-idle
    Activation engine via Square+accum_out, entirely off the DVE.

Layout: m-major bf16. Host repacks dense to [B, (m=64, n=32)] bf16 (halves
HBM traffic; all DVE tensor ops become 2-byte -> 2x DVE rate) and keeps a
separate f32 [B, 32] copy of dense[:, :, 0] for the numerically dominant
linear term. The FM term is ~1e-3 of the output, so bf16 there is safe.

Sharding: pure data parallel, batch 4096 -> 512 rows on each of 8 cores,
4 tiles of 128 rows.

Per-core engine assignment:
  SYNC: 8 half-tile dense loads (queued immediately, FIFO rings keep
        completion order = issue order) + one [128,4] output store.
  ACT:  param loads on its own HWDGE ring (parallel with SYNC's issue);
        per tile: two Square+accum_out ops over the sign blocks of y
        (the whole S2 path); a warmup square triggers ACT_TABLE_LOAD early.
  DVE:  per tile: y = d*u in two bf16 halves (2x mode), 5-level bf16
        pairwise add-tree reducing n within each m group -> S1 [128, 64] f32,
        two TENSOR_TENSOR_REDUCE ops (+-1) turning S1 into the signed
        sum of squares (seeded with linear+bias), and one
        scalar_tensor_tensor merge per tile folding in the ACT accums.
        The linear term is computed once for all 4 tiles (mul + grouped
        tensor_reduce over a [128, 4, 32] f32 pack of dense[:, :, 0]).
"""

import numpy as np

B, N, M = 4096, 32, 64
NM = N * M                  # 2048
HALF = NM // 2              # 1024
NCORES = 8
BS = B // NCORES            # 512 rows per core
TILES = BS // 128           # 4 tiles of 128 batch rows per core
P_PAIRS = N * (N - 1) // 2  # 496

_CACHE = {}


def _build_program(K):
    """K = number of m columns with c >= 0 (they are packed first)."""
    from concourse import bacc, mybir
    from concourse.dve_ops import TENSOR_TENSOR_REDUCE as CTTR

    f32 = mybir.dt.float32
    bf16 = mybir.dt.bfloat16
    Square = mybir.ActivationFunctionType.Square
    sub = mybir.AluOpType.subtract
    add = mybir.AluOpType.add

    nc = bacc.Bacc("TRN2", target_bir_lowering=False, debug=False)
    dense = nc.declare_dram_parameter("dense", [BS, NM], bf16, isOutput=False)
    urep = nc.declare_dram_parameter("urep", [128, NM], bf16, isOutput=False)
    spd = nc.declare_dram_parameter("spd", [128, TILES * N], f32, isOutput=False)
    wlrep4 = nc.declare_dram_parameter("wlrep4", [128, TILES * N], f32, isOutput=False)
    cst = nc.declare_dram_parameter("cst", [128, 1], f32, isOutput=False)
    out = nc.declare_dram_parameter("out", [128, TILES], f32, isOutput=True)

    sb = lambda name, shape, dt: nc.alloc_sbuf_tensor(name, list(shape), dt)

    urep_t = sb("urep_t", [128, NM], bf16)
    spd_t = sb("spd_t", [128, TILES * N], f32)
    wlrep4_t = sb("wlrep4_t", [128, TILES * N], f32)
    cst_t = sb("cst_t", [128, 1], f32)
    spw_t = sb("spw_t", [128, TILES * N], f32)
    lin4_t = sb("lin4_t", [128, TILES], f32)
    seed4_t = sb("seed4_t", [128, TILES], f32)
    o2all = sb("o2all", [128, TILES], f32)
    warm_t = sb("warm_t", [128, 1], f32)
    y2j = sb("y2j", [128, NM], bf16)       # ACT square junk output
    junk = sb("junk", [128, M], f32)       # DVE CTTR junk output

    df_t, y_t, l_t, s1_t = [], [], [], []
    a1_t, a2_t, sqp_t, sqn_t = [], [], [], []
    for t in range(TILES):
        df_t.append(sb(f"df{t}", [128, NM], bf16))
        y_t.append(sb(f"y{t}", [128, NM], bf16))
        l_t.append([sb(f"l{t}_{w}", [128, M * w], bf16) for w in (16, 8, 4, 2)])
        s1_t.append(sb(f"s1_{t}", [128, M], f32))
        a1_t.append(sb(f"a1_{t}", [128, 1], f32))
        a2_t.append(sb(f"a2_{t}", [128, 1], f32))
        sqp_t.append(sb(f"sqp_{t}", [128, 1], f32))
        sqn_t.append(sb(f"sqn_{t}", [128, 1], f32))

    cnt = {"v": 0, "a": 0, "s": 0}
    chains = {}

    def emit(e, ins):
        ins._wait_ge(chains[e], cnt[e]).then_inc(chains[e], 1)
        cnt[e] += 1
        return cnt[e]

    def emit_dma(e, ins, sem, inc, wait=None):
        if wait is not None:
            wsem, wval = wait
            ins._wait_ge(wsem, wval)
        else:
            ins._wait_ge(chains[e], cnt[e])
        ins.then_inc(sem, inc)

    def emit_wait(e, eng, sem, val):
        eng.wait_ge(sem, val).then_inc(chains[e], 1)
        cnt[e] += 1

    # sign blocks as (start, width, sign) over the m axis, skipping empties
    blocks = [(0, K, 1.0), (K, M - K, -1.0)]
    blocks = [b for b in blocks if b[1] > 0]

    # ACT chain values after tile t's squares (scalar block is built after
    # the vector block, so predict its chain; asserted below)
    n_sq = len(blocks)
    ach_sq_done = [2 + n_sq * (t + 1) + (t + 1) for t in range(TILES)]

    mulB_done = [0] * TILES
    o2_done = [0]

    with (
        nc.Block() as block,
        nc.semaphore("vch") as vch,
        nc.semaphore("ach") as ach,
        nc.semaphore("sch") as sch,
        nc.semaphore("ld0a") as ld0a,
        nc.semaphore("ld0b") as ld0b,
        nc.semaphore("ld1a") as ld1a,
        nc.semaphore("ld1b") as ld1b,
        nc.semaphore("ld2a") as ld2a,
        nc.semaphore("ld2b") as ld2b,
        nc.semaphore("ld3a") as ld3a,
        nc.semaphore("ld3b") as ld3b,
        nc.semaphore("prm") as prm,
        nc.semaphore("sts") as sts,
    ):
        chains.update(v=vch, a=ach, s=sch)
        lda = [ld0a, ld1a, ld2a, ld3a]
        ldb = [ld0b, ld1b, ld2b, ld3b]

        @block.vector
        def _(dve):
            def muls(t):
                emit_wait("v", dve, lda[t], 16)
                emit("v", dve.tensor_mul(
                    y_t[t].ap()[:, 0:HALF], df_t[t].ap()[:, 0:HALF],
                    urep_t.ap()[:, 0:HALF],
                ))
                emit_wait("v", dve, ldb[t], 16)
                mulB_done[t] = emit("v", dve.tensor_mul(
                    y_t[t].ap()[:, HALF:NM], df_t[t].ap()[:, HALF:NM],
                    urep_t.ap()[:, HALF:NM],
                ))

            def tree(t):
                # pairwise halving of n (32->16->8->4->2->1) inside each of
                # the 64 m groups; all-bf16 strided views stay in 2x mode
                src = y_t[t].ap().rearrange("p (m n) -> p m n", m=M)
                w = N
                for lv in range(4):
                    w //= 2
                    dst = l_t[t][lv].ap().rearrange("p (m n) -> p m n", m=M)
                    emit("v", dve.tensor_add(
                        dst, src[:, :, 0:w], src[:, :, w : 2 * w]
                    ))
                    src = dst
                emit("v", dve.tensor_add(
                    s1_t[t].ap().rearrange("p (m n) -> p m n", n=1),
                    src[:, :, 0:1], src[:, :, 1:2],
                ))

            def cttrs(t):
                seed = seed4_t.ap()[:, t : t + 1]
                accs = (a1_t[t], a2_t[t])
                for i, (m0, mw, sg) in enumerate(blocks):
                    sl = s1_t[t].ap()[:, m0 : m0 + mw]
                    emit("v", dve._custom_dve(
                        CTTR, out=junk.ap()[:, 0:mw], in0=sl, in1=sl,
                        s0=seed, s1=sg, accum_out=accs[i].ap(),
                    ))
                    seed = accs[i].ap()
                return seed  # [p,1] = seed4 + sum_pos S1^2 - sum_neg S1^2

            def merge(t, a_last):
                emit_wait("v", dve, ach, ach_sq_done[t])
                o2col = o2all.ap()[:, t : t + 1]
                if len(blocks) == 2:
                    # o2 = (sqn - sqp) + a_last
                    emit("v", dve.scalar_tensor_tensor(
                        out=o2col, in0=sqn_t[t].ap(), scalar=sqp_t[t].ap(),
                        in1=a_last, op0=sub, op1=add,
                    ))
                elif blocks[0][2] > 0:  # all positive: o2 = a_last - sqp
                    emit("v", dve.tensor_sub(o2col, a_last, sqp_t[t].ap()))
                else:                   # all negative: o2 = a_last + sqn
                    emit("v", dve.tensor_add(o2col, a_last, sqn_t[t].ap()))

            emit_wait("v", dve, prm, 32)       # urep halves landed
            muls(0)
            emit_wait("v", dve, prm, 80)       # spd/wlrep4/cst landed
            # linear term for all 4 tiles: lin4[p,t] = sum_n spd[p,t,n]*Wl[n]
            emit("v", dve.tensor_mul(spw_t.ap(), spd_t.ap(), wlrep4_t.ap()))
            emit("v", dve.tensor_reduce(
                lin4_t.ap(),
                spw_t.ap().rearrange("p (t n) -> p t n", t=TILES),
                axis=mybir.AxisListType.X, op=add,
            ))
            emit("v", dve.tensor_scalar_add(seed4_t.ap(), lin4_t.ap(), cst_t.ap()))

            a_last = [None] * TILES
            tree(0)
            a_last[0] = cttrs(0)
            for t in range(1, TILES):
                muls(t)
                tree(t)
                a_last[t] = cttrs(t)
                merge(t - 1, a_last[t - 1])
            merge(TILES - 1, a_last[TILES - 1])
            o2_done[0] = cnt["v"]

        @block.scalar
        def _(act):
            # param loads ride the Activation HWDGE ring, issuing in
            # parallel with the dense loads on the SP ring
            emit_dma("a", act.dma_start(
                out=urep_t.ap()[:, 0:HALF], in_=urep.ap()[:, 0:HALF]), prm, 16)
            emit_dma("a", act.dma_start(
                out=urep_t.ap()[:, HALF:NM], in_=urep.ap()[:, HALF:NM]), prm, 16)
            emit_dma("a", act.dma_start(out=spd_t.ap(), in_=spd.ap()), prm, 16)
            emit_dma("a", act.dma_start(out=wlrep4_t.ap(), in_=wlrep4.ap()), prm, 16)
            emit_dma("a", act.dma_start(out=cst_t.ap(), in_=cst.ap()), prm, 16)
            emit_wait("a", act, prm, 80)
            # warmup: trigger the one-time ACT_TABLE_LOAD during the DMA lead-in
            emit("a", act.square(warm_t.ap(), cst_t.ap()))
            for t in range(TILES):
                emit_wait("a", act, vch, mulB_done[t])
                accs = (sqp_t[t], sqn_t[t]) if blocks[0][2] > 0 else (sqn_t[t],)
                for i, (m0, mw, sg) in enumerate(blocks):
                    lo, hi = m0 * N, (m0 + mw) * N
                    emit("a", act.activation(
                        out=y2j.ap()[:, lo:hi], in_=y_t[t].ap()[:, lo:hi],
                        func=Square, accum_out=accs[i].ap(),
                    ))
            assert cnt["a"] == ach_sq_done[-1], (cnt["a"], ach_sq_done)

        @block.sync
        def _(sync):
            for t in range(TILES):
                for h, sem in ((0, lda[t]), (1, ldb[t])):
                    lo, hi = (0, HALF) if h == 0 else (HALF, NM)
                    emit_dma("s", sync.dma_start(
                        out=df_t[t].ap()[:, lo:hi],
                        in_=dense.ap()[128 * t : 128 * (t + 1), lo:hi],
                    ), sem, 16)
            emit_dma("s", sync.dma_start(out=out.ap(), in_=o2all.ap()),
                     sts, 16, wait=(vch, o2_done[0]))
            sync.wait_ge(sts, 16)

    nc.compile()
    return nc


def _get_program(K):
    if K not in _CACHE:
        _CACHE[K] = _build_program(K)
    return _CACHE[K]


def _host_prep(inputs):
    import ml_dtypes

    dense = np.asarray(inputs["dense"], dtype=np.float32)  # [B, N, M]
    v = np.asarray(inputs["v"], dtype=np.float32)          # [N, M]
    Wl = np.asarray(inputs["Wl"], dtype=np.float32).reshape(N)
    Wp = np.asarray(inputs["Wp"], dtype=np.float32).reshape(M)
    bl = float(np.asarray(inputs["bl"], dtype=np.float32).reshape(-1)[0])
    bp = float(np.asarray(inputs["bp"], dtype=np.float32).reshape(-1)[0])

    c = (Wp / (2.0 * P_PAIRS)).astype(np.float32)
    pos = np.where(c >= 0)[0]
    neg = np.where(c < 0)[0]
    idx = np.concatenate([pos, neg])
    K = int(len(pos))

    # m-major, sign-sorted, sqrt|c|-scaled replica of v -> u [64, 32]
    u = (v * np.sqrt(np.abs(c))[None, :]).T[idx]               # [M, N]
    urep = np.ascontiguousarray(np.broadcast_to(
        u.reshape(1, NM).astype(ml_dtypes.bfloat16), (128, NM)))

    # dense repacked m-major + sign-sorted: [B, (m, n)] bf16
    dmm = np.ascontiguousarray(
        dense.transpose(0, 2, 1)[:, idx, :].reshape(B, NM)
    ).astype(ml_dtypes.bfloat16)

    sparse = np.ascontiguousarray(dense[:, :, 0])              # [B, N] f32
    wlrep4 = np.ascontiguousarray(np.broadcast_to(
        np.tile(Wl, TILES)[None, :], (128, TILES * N)))
    cstv = np.full((128, 1), bl + bp, dtype=np.float32)

    in_maps = []
    for i in range(NCORES):
        spd = np.ascontiguousarray(
            sparse[BS * i : BS * (i + 1)]
            .reshape(TILES, 128, N).transpose(1, 0, 2).reshape(128, TILES * N)
        )
        in_maps.append({
            "dense": dmm[BS * i : BS * (i + 1)],
            "urep": urep,
            "spd": spd,
            "wlrep4": wlrep4,
            "cst": cstv,
        })
    return K, in_maps


def _gather(res):
    # out[p, t] holds batch row 128*t + p of the core's shard
    outs = []
    for i in range(NCORES):
        arr = np.asarray(res.results[i]["out"], np.float32)  # [128, TILES]
        outs.append(arr.T.reshape(BS))
    return np.concatenate(outs).reshape(B, 1)


def kernel(**inputs) -> np.ndarray:
    from concourse.bass_utils import run_bass_kernel_spmd

    K, in_maps = _host_prep(inputs)
    nc = _get_program(K)
    res = run_bass_kernel_spmd(nc, in_maps, core_ids=list(range(NCORES)))
    return _gather(res)


# revision 10
# speedup vs baseline: 1.2672x; 1.0318x over previous
"""Trainium2 Bass kernel for nn_AFM (attentional factorization machine).

Mathematical reduction (validated against the reference):
  - softmax over a size-1 axis == 1, so the attention MLP is dead code and
    fAtt = mean(fPI, axis=1).
  - FM identity per (b, m): sum_{i<j} x_i x_j = ((sum_i x_i)^2 - sum_i x_i^2)/2
    with x_i = dense[b,i,m] * v[i,m].
  - Sign-split scaling: with c[m] = Wp[m]/(2P), u[n,m] = v[n,m]*sqrt(|c[m]|)
    and y = d*u, the FM term becomes
      sum_m sign(c[m]) * ((sum_n y)^2 - sum_n y^2).
    Host reorders the m axis so all c>=0 columns come first (K of them);
    then sum_m sign*(sum_n y^2) collapses to TWO plain free-axis sums of y^2
    (one per contiguous sign block) - computed on the otherwise-idle
    Activation engine via Square+accum_out, entirely off the DVE.

Layout: m-major bf16. Host repacks dense to [B, (m=64, n=32)] bf16 (halves
HBM traffic; all DVE tensor ops become 2-byte -> 2x DVE rate) and keeps a
separate f32 [B, 32] copy of dense[:, :, 0] for the numerically dominant
linear term. The FM term is ~1e-3 of the output, so bf16 there is safe.

Sharding: pure data parallel, batch 4096 -> 512 rows on each of 8 cores,
4 tiles of 128 rows.

Per-core engine assignment:
  SYNC: dense loads (tile 0 in four quarter-tile chunks so compute starts
        ~3us earlier; tiles 1-3 in halves), all queued immediately - the
        HWDGE rings are FIFO so completion order = issue order.
  ACT:  param loads on its own HWDGE ring (urep in quarters, first, so the
        first quarter-mul only waits on 128KB of params); per tile two
        Square+accum_out ops over the sign blocks of y (the whole S2 path);
        a warmup square triggers the one-time ACT_TABLE_LOAD early.
  DVE:  per tile: y = d*u (bf16 2x mode; tile 0 as 4 quarter muls, rest as
        one full-tile mul), S1 via 2 pairwise bf16 add-tree levels plus one
        grouped tensor_reduce (axis=X) -> S1 [128, 64] f32, two
        TENSOR_TENSOR_REDUCE ops (+-1, seeded with linear+bias) for the
        signed sum of S1^2, one scalar_tensor_tensor merge per tile, and
        the final [128, 4] output store from the DVE's own DGE ring (saves
        the 900ns DMA-sem hop to SYNC).
"""

import numpy as np

B, N, M = 4096, 32, 64
NM = N * M                  # 2048
HALF = NM // 2              # 1024
QTR = NM // 4               # 512
NCORES = 8
BS = B // NCORES            # 512 rows per core
TILES = BS // 128           # 4 tiles of 128 batch rows per core
P_PAIRS = N * (N - 1) // 2  # 496

_CACHE = {}


def _build_program(K):
    """K = number of m columns with c >= 0 (they are packed first)."""
    from concourse import bacc, mybir
    from concourse.dve_ops import TENSOR_TENSOR_REDUCE as CTTR

    f32 = mybir.dt.float32
    bf16 = mybir.dt.bfloat16
    Square = mybir.ActivationFunctionType.Square
    sub = mybir.AluOpType.subtract
    add = mybir.AluOpType.add

    nc = bacc.Bacc("TRN2", target_bir_lowering=False, debug=False)
    dense = nc.declare_dram_parameter("dense", [BS, NM], bf16, isOutput=False)
    urep = nc.declare_dram_parameter("urep", [128, NM], bf16, isOutput=False)
    spd = nc.declare_dram_parameter("spd", [128, TILES * N], f32, isOutput=False)
    wlrep4 = nc.declare_dram_parameter("wlrep4", [128, TILES * N], f32, isOutput=False)
    cst = nc.declare_dram_parameter("cst", [128, 1], f32, isOutput=False)
    out = nc.declare_dram_parameter("out", [128, TILES], f32, isOutput=True)

    sb = lambda name, shape, dt: nc.alloc_sbuf_tensor(name, list(shape), dt)

    urep_t = sb("urep_t", [128, NM], bf16)
    spd_t = sb("spd_t", [128, TILES * N], f32)
    wlrep4_t = sb("wlrep4_t", [128, TILES * N], f32)
    cst_t = sb("cst_t", [128, 1], f32)
    spw_t = sb("spw_t", [128, TILES * N], f32)
    lin4_t = sb("lin4_t", [128, TILES], f32)
    seed4_t = sb("seed4_t", [128, TILES], f32)
    o2all = sb("o2all", [128, TILES], f32)
    warm_t = sb("warm_t", [128, 1], f32)
    y2j = sb("y2j", [128, NM], bf16)       # ACT square junk output
    junk = sb("junk", [128, M], f32)       # DVE CTTR junk output

    df_t, y_t, l0_t, l1_t, s1_t = [], [], [], [], []
    a1_t, a2_t, sqp_t, sqn_t = [], [], [], []
    for t in range(TILES):
        df_t.append(sb(f"df{t}", [128, NM], bf16))
        y_t.append(sb(f"y{t}", [128, NM], bf16))
        l0_t.append(sb(f"l0_{t}", [128, M * 16], bf16))
        l1_t.append(sb(f"l1_{t}", [128, M * 8], bf16))
        s1_t.append(sb(f"s1_{t}", [128, M], f32))
        a1_t.append(sb(f"a1_{t}", [128, 1], f32))
        a2_t.append(sb(f"a2_{t}", [128, 1], f32))
        sqp_t.append(sb(f"sqp_{t}", [128, 1], f32))
        sqn_t.append(sb(f"sqn_{t}", [128, 1], f32))

    cnt = {"v": 0, "a": 0, "s": 0}
    chains = {}

    def emit(e, ins):
        ins._wait_ge(chains[e], cnt[e]).then_inc(chains[e], 1)
        cnt[e] += 1
        return cnt[e]

    def emit_dma(e, ins, sem, inc, wait=None):
        if wait is not None:
            wsem, wval = wait
            ins._wait_ge(wsem, wval)
        else:
            ins._wait_ge(chains[e], cnt[e])
        ins.then_inc(sem, inc)

    def emit_wait(e, eng, sem, val):
        eng.wait_ge(sem, val).then_inc(chains[e], 1)
        cnt[e] += 1

    # sign blocks as (start, width, sign) over the m axis, skipping empties
    blocks = [(0, K, 1.0), (K, M - K, -1.0)]
    blocks = [b for b in blocks if b[1] > 0]

    # ACT chain values after tile t's squares (scalar block is built after
    # the vector block, so predict its chain; asserted below)
    n_sq = len(blocks)
    ach_sq_done = [2 + n_sq * (t + 1) + (t + 1) for t in range(TILES)]

    mulB_done = [0] * TILES
    o2_done = [0]

    # NOTE on DMA semaphores: a dma_start's +16 completion budget is spread
    # over its descriptors, and descriptors of LATER starts on the shared
    # queues can complete before an earlier start's last descriptor. So a
    # shared semaphore only safely gates at its FULL total; every load that
    # gates compute at an intermediate point gets its own semaphore.
    with (
        nc.Block() as block,
        nc.semaphore("vch") as vch,
        nc.semaphore("ach") as ach,
        nc.semaphore("sch") as sch,
        nc.semaphore("ldq0") as ldq0,
        nc.semaphore("ldq1") as ldq1,
        nc.semaphore("ldq2") as ldq2,
        nc.semaphore("ldq3") as ldq3,
        nc.semaphore("ld1") as ld1,
        nc.semaphore("ld2") as ld2,
        nc.semaphore("ld3") as ld3,
        nc.semaphore("uq0") as uq0,
        nc.semaphore("uq1") as uq1,
        nc.semaphore("uq2") as uq2,
        nc.semaphore("uq3") as uq3,
        nc.semaphore("prm") as prm,
        nc.semaphore("sts") as sts,
    ):
        chains.update(v=vch, a=ach, s=sch)
        ldq = [ldq0, ldq1, ldq2, ldq3]
        ld = [None, ld1, ld2, ld3]
        uq = [uq0, uq1, uq2, uq3]

        @block.vector
        def _(dve):
            def mul_range(t, lo, hi):
                return emit("v", dve.tensor_mul(
                    y_t[t].ap()[:, lo:hi], df_t[t].ap()[:, lo:hi],
                    urep_t.ap()[:, lo:hi],
                ))

            def tree(t):
                # n halves 32->16->8 inside each m group (bf16 2x mode),
                # then one grouped reduce [p, 64, 8] -> [p, 64] f32
                src = y_t[t].ap().rearrange("p (m n) -> p m n", m=M)
                d0 = l0_t[t].ap().rearrange("p (m n) -> p m n", m=M)
                emit("v", dve.tensor_add(d0, src[:, :, 0:16], src[:, :, 16:32]))
                d1 = l1_t[t].ap().rearrange("p (m n) -> p m n", m=M)
                emit("v", dve.tensor_add(d1, d0[:, :, 0:8], d0[:, :, 8:16]))
                emit("v", dve.tensor_reduce(
                    s1_t[t].ap(), d1, axis=mybir.AxisListType.X, op=add,
                ))

            def cttrs(t):
                seed = seed4_t.ap()[:, t : t + 1]
                accs = (a1_t[t], a2_t[t])
                for i, (m0, mw, sg) in enumerate(blocks):
                    sl = s1_t[t].ap()[:, m0 : m0 + mw]
                    emit("v", dve._custom_dve(
                        CTTR, out=junk.ap()[:, 0:mw], in0=sl, in1=sl,
                        s0=seed, s1=sg, accum_out=accs[i].ap(),
                    ))
                    seed = accs[i].ap()
                return seed  # [p,1] = seed4 + sum_pos S1^2 - sum_neg S1^2

            def merge(t, a_last):
                emit_wait("v", dve, ach, ach_sq_done[t])
                o2col = o2all.ap()[:, t : t + 1]
                if len(blocks) == 2:
                    # o2 = (sqn - sqp) + a_last
                    emit("v", dve.scalar_tensor_tensor(
                        out=o2col, in0=sqn_t[t].ap(), scalar=sqp_t[t].ap(),
                        in1=a_last, op0=sub, op1=add,
                    ))
                elif blocks[0][2] > 0:  # all positive: o2 = a_last - sqp
                    emit("v", dve.tensor_sub(o2col, a_last, sqp_t[t].ap()))
                else:                   # all negative: o2 = a_last + sqn
                    emit("v", dve.tensor_add(o2col, a_last, sqn_t[t].ap()))

            # tile 0 in quarters so compute starts on the first 256KB
            for q in range(4):
                emit_wait("v", dve, uq[q], 16)
                emit_wait("v", dve, ldq[q], 16)
                r = mul_range(0, QTR * q, QTR * (q + 1))
            mulB_done[0] = r
            emit_wait("v", dve, prm, 48)       # spd/wlrep4/cst landed
            # linear term for all 4 tiles: lin4[p,t] = sum_n spd[p,t,n]*Wl[n]
            emit("v", dve.tensor_mul(spw_t.ap(), spd_t.ap(), wlrep4_t.ap()))
            emit("v", dve.tensor_reduce(
                lin4_t.ap(),
                spw_t.ap().rearrange("p (t n) -> p t n", t=TILES),
                axis=mybir.AxisListType.X, op=add,
            ))
            emit("v", dve.tensor_scalar_add(seed4_t.ap(), lin4_t.ap(), cst_t.ap()))

            a_last = [None] * TILES
            tree(0)
            a_last[0] = cttrs(0)
            for t in range(1, TILES):
                emit_wait("v", dve, ld[t], 32)
                mulB_done[t] = mul_range(t, 0, NM)
                tree(t)
                a_last[t] = cttrs(t)
                merge(t - 1, a_last[t - 1])
            merge(TILES - 1, a_last[TILES - 1])
            o2_done[0] = cnt["v"]

        @block.scalar
        def _(act):
            # param loads ride the Activation HWDGE ring, issuing in
            # parallel with the dense loads on the SP ring
            for q in range(4):
                emit_dma("a", act.dma_start(
                    out=urep_t.ap()[:, QTR * q : QTR * (q + 1)],
                    in_=urep.ap()[:, QTR * q : QTR * (q + 1)]), uq[q], 16)
            emit_dma("a", act.dma_start(out=spd_t.ap(), in_=spd.ap()), prm, 16)
            emit_dma("a", act.dma_start(out=wlrep4_t.ap(), in_=wlrep4.ap()), prm, 16)
            emit_dma("a", act.dma_start(out=cst_t.ap(), in_=cst.ap()), prm, 16)
            emit_wait("a", act, prm, 48)
            # warmup: trigger the one-time ACT_TABLE_LOAD during the DMA lead-in
            emit("a", act.square(warm_t.ap(), cst_t.ap()))
            for t in range(TILES):
                emit_wait("a", act, vch, mulB_done[t])
                accs = (sqp_t[t], sqn_t[t]) if blocks[0][2] > 0 else (sqn_t[t],)
                for i, (m0, mw, sg) in enumerate(blocks):
                    lo, hi = m0 * N, (m0 + mw) * N
                    emit("a", act.activation(
                        out=y2j.ap()[:, lo:hi], in_=y_t[t].ap()[:, lo:hi],
                        func=Square, accum_out=accs[i].ap(),
                    ))
            assert cnt["a"] == ach_sq_done[-1], (cnt["a"], ach_sq_done)
            # output store from the ACT ring (idle by now), gated on merge3;
            # engine-to-engine sem hop is ~100ns vs 900ns for DMA-completion
            emit_dma("a", act.dma_start(out=out.ap(), in_=o2all.ap()),
                     sts, 16, wait=(vch, o2_done[0]))

        @block.sync
        def _(sync):
            for q in range(4):  # tile 0 quarters
                emit_dma("s", sync.dma_start(
                    out=df_t[0].ap()[:, QTR * q : QTR * (q + 1)],
                    in_=dense.ap()[0:128, QTR * q : QTR * (q + 1)],
                ), ldq[q], 16)
            for t in range(1, TILES):
                for h in range(2):
                    lo, hi = (0, HALF) if h == 0 else (HALF, NM)
                    emit_dma("s", sync.dma_start(
                        out=df_t[t].ap()[:, lo:hi],
                        in_=dense.ap()[128 * t : 128 * (t + 1), lo:hi],
                    ), ld[t], 16)
            sync.wait_ge(sts, 16)

    nc.compile()
    return nc


def _get_program(K):
    if K not in _CACHE:
        _CACHE[K] = _build_program(K)
    return _CACHE[K]


def _host_prep(inputs):
    import ml_dtypes

    dense = np.asarray(inputs["dense"], dtype=np.float32)  # [B, N, M]
    v = np.asarray(inputs["v"], dtype=np.float32)          # [N, M]
    Wl = np.asarray(inputs["Wl"], dtype=np.float32).reshape(N)
    Wp = np.asarray(inputs["Wp"], dtype=np.float32).reshape(M)
    bl = float(np.asarray(inputs["bl"], dtype=np.float32).reshape(-1)[0])
    bp = float(np.asarray(inputs["bp"], dtype=np.float32).reshape(-1)[0])

    c = (Wp / (2.0 * P_PAIRS)).astype(np.float32)
    pos = np.where(c >= 0)[0]
    neg = np.where(c < 0)[0]
    idx = np.concatenate([pos, neg])
    K = int(len(pos))

    # m-major, sign-sorted, sqrt|c|-scaled replica of v -> u [64, 32]
    u = (v * np.sqrt(np.abs(c))[None, :]).T[idx]               # [M, N]
    urep = np.ascontiguousarray(np.broadcast_to(
        u.reshape(1, NM).astype(ml_dtypes.bfloat16), (128, NM)))

    # dense repacked m-major + sign-sorted: [B, (m, n)] bf16
    dmm = np.ascontiguousarray(
        dense.transpose(0, 2, 1)[:, idx, :].reshape(B, NM)
    ).astype(ml_dtypes.bfloat16)

    sparse = np.ascontiguousarray(dense[:, :, 0])              # [B, N] f32
    wlrep4 = np.ascontiguousarray(np.broadcast_to(
        np.tile(Wl, TILES)[None, :], (128, TILES * N)))
    cstv = np.full((128, 1), bl + bp, dtype=np.float32)

    in_maps = []
    for i in range(NCORES):
        spdi = np.ascontiguousarray(
            sparse[BS * i : BS * (i + 1)]
            .reshape(TILES, 128, N).transpose(1, 0, 2).reshape(128, TILES * N)
        )
        in_maps.append({
            "dense": dmm[BS * i : BS * (i + 1)],
            "urep": urep,
            "spd": spdi,
            "wlrep4": wlrep4,
            "cst": cstv,
        })
    return K, in_maps


def _gather(res):
    # out[p, t] holds batch row 128*t + p of the core's shard
    outs = []
    for i in range(NCORES):
        arr = np.asarray(res.results[i]["out"], np.float32)  # [128, TILES]
        outs.append(arr.T.reshape(BS))
    return np.concatenate(outs).reshape(B, 1)


def kernel(**inputs) -> np.ndarray:
    from concourse.bass_utils import run_bass_kernel_spmd

    K, in_maps = _host_prep(inputs)
    nc = _get_program(K)
    res = run_bass_kernel_spmd(nc, in_maps, core_ids=list(range(NCORES)))
    return _gather(res)
